# revision 72
# baseline (speedup 1.0000x reference)
"""BaiChuan attention layer on 8 Trainium2 NeuronCores.

Sharding: tensor-parallel over heads across all 8 cores (4 heads per
core, both batches on every core).  o_proj is token-parallel: per-head
attention outputs are exchanged with one 8-way AllToAll per causal
token-half, after which each core computes the full o_proj for its
(batch, token-quarter) share with w_o streamed from HBM.

Precision: q/k projection runs in fp8e4 with DoubleRow (inputs scaled by
64, descaled via the rope cos/sin tables which are pre-divided by 4096);
everything else is bf16 operands with f32 PSUM accumulation.  Softmax
denominator accumulates on the vector engine (bf16, 2x mode) and is
partition-summed with one ones-matmul per attention block.  The output
leaves the device in bf16 and is upcast on the host.

Pipeline (core c: heads 4c..4c+4 of both batches; owns output tokens
[256*(c%4), +256) and [1024+256*(c%4), +256) of batch c//4):
  stage A  (8 token blocks of 512 over batch0++batch1): qkvT[j,t];
           v in bf16, q/k in fp8 DoubleRow.  hs inputs are host-packed
           into a blocked layout so every load is one contiguous 16KB
           run per partition; wv/w8 weights are SBUF-resident (loaded
           once, tags recycled by stage C's w_o stream); pass-1 k goes
           straight into persistent SBUF tiles (no DRAM round-trip).
           Only blocks 0,1,4,5 are emitted up front: blocks 2,3 / 6,7
           are woven into pass 1 / pass 2 as 4-matmul fill chunks
           (Filler) so the PE stays dense through the attention phases.
  pass 1   per head-instance: RoPE (k in place, q via DRAM round-trip),
           causal attention for q in [0,1024).  The score->exp->attn@v
           chain is pipelined two score-batches deep so the PE never
           waits on the exp->mask latency.
  A2A #1   8-way AllToAll of all instances' attn[:, 0:1024].
  pass 2   attention for q in [1024,2048) (resident roped k for the
           first token half, only [1024:2048) re-roped).
  A2A #2   second half exchange.
  stage C  o_proj per token quarter: out[t, m] over all 32 heads, w_o
           streamed per 512-wide m block; half-0's at/wo tiles are
           prefetched during pass 2 (collective-dependent loads ride
           the otherwise-idle Act DGE queue so a parked descriptor
           never blocks the SP DMA rings).
"""
import sys
sys.path.insert(0, '/opt/trn_rl_repo')
import numpy as np
import ml_dtypes

import concourse.bass as bass
from concourse import bacc
import concourse.mybir as mybir
from concourse.tile import TileContext
from concourse.bass_utils import run_bass_kernel_spmd
from concourse.masks import make_identity

f32 = mybir.dt.float32
bf16 = mybir.dt.bfloat16
fp8 = mybir.dt.float8e4
AF = mybir.ActivationFunctionType
DR = mybir.MatmulPerfMode.DoubleRowSwInterleave

B, S, H, NH = 2, 2048, 4096, 32
HD = H // NH                    # 128
THETA = 10000.0
NCORES = 8
HPC = NH // NCORES              # 4 heads per core
NI = B * HPC                    # 8 head-instances (batch, head) per core
JCC = HPC * HD                  # 512 = per-core head width (one batch)
SCALE = HD ** -0.5
SCALEQK = 64.0                  # fp8 input scaling for the q/k projection
GROUPS = [[0, 1, 2, 3, 4, 5, 6, 7]]
TB = 512                        # stage-A token block
NTB = B * S // TB               # 8 blocks over batch0 ++ batch1
NIB = H // 128                  # 32 contraction blocks
TH = S // 2                     # causal token half
QT = 256                        # tokens per (core, half)
NIB2 = NIB // 2
NJB = NH                        # 32 o_proj j-blocks


def build_nc():
    nc = bacc.Bacc(None)
    # blocked hs layouts: [128 p, (tb n), t] so stage-A loads are
    # contiguous 16KB runs per partition
    hs16d = nc.declare_dram_parameter("hs16d", [128, NTB * NIB, TB], bf16,
                                      isOutput=False)
    hs8d = nc.declare_dram_parameter("hs8d", [128, NTB * NIB, TB], fp8,
                                     isOutput=False)
    # SwInterleave stationary blocks: [128 p, ob*16+ibpair, 256 interleaved]
    # ob = kq out-block (hh*2 + part) of this core, 8 per core
    wT8 = nc.declare_dram_parameter("wT8", [128, 8 * NIB2, 256], fp8,
                                    isOutput=False)
    wTv = nc.declare_dram_parameter("wTv", [H, JCC], bf16, isOutput=False)
    woT = nc.declare_dram_parameter("woT", [H, H], bf16, isOutput=False)
    cosf = nc.declare_dram_parameter("cosf", [HD, S], bf16, isOutput=False)
    sinm = nc.declare_dram_parameter("sinm", [HD, S], bf16, isOutput=False)
    masks = nc.declare_dram_parameter("masks", [4, 128, 512], bf16,
                                      isOutput=False)
    # output leaves the device in bf16 (host upcasts); halves the stage-C
    # store traffic and the end-of-kernel drain
    out = nc.declare_dram_parameter("out", [2 * QT, H], bf16, isOutput=True)

    # per head-instance i = b*HPC + hh: k/q at kq_d[2i + 0/1]
    kq_d = [nc.dram_tensor(f"kq_d{j}", [128, S], bf16)
            for j in range(2 * NI)]
    # v in natural [t, j] layout, one tensor per batch
    v_d = [nc.dram_tensor(f"v_d{b}", [S, JCC], bf16) for b in range(B)]
    a2a_in = [nc.dram_tensor(f"a2a{i}_in", [NCORES, JCC, QT], bf16)
              for i in range(2)]
    a2a_out = [nc.dram_tensor(f"a2a{i}_out", [NCORES, JCC, QT], bf16)
               for i in range(2)]

    wTv_v = wTv[:].rearrange("(n p) j -> p n j", p=128)       # [128, 32, 512]
    v_d_v = [t[:].rearrange("(kb p) j -> p kb j", p=128)      # [128, 16, 512]
             for t in v_d]
    woT_v = woT[:].rearrange("(n p) m -> p n m", p=128)       # [128, 32, H]
    at_v = [t[:].rearrange("r (n p) t -> p (r n) t", p=128)   # [128, 32, QT]
            for t in a2a_out]

    with TileContext(nc) as tc:
        with tc.tile_pool(name="const", bufs=1) as pconst, \
             tc.tile_pool(name="stA", bufs=1) as pa, \
             tc.tile_pool(name="stB", bufs=1) as pb, \
             tc.tile_pool(name="stC", bufs=1) as pc, \
             tc.tile_pool(name="psum", bufs=1, space="PSUM") as ps:
            # ---------------- constants (PE warmup first) ----------------
            ident = pconst.tile([128, 128], bf16, tag="ident", bufs=1)
            make_identity(nc, ident[:])
            ones16 = pconst.tile([128, 128], bf16, tag="ones16", bufs=1)
            nc.vector.memset(ones16[:], 1.0)
            # PE warmup burst (HAM un-throttle while first DMAs land)
            for wu in range(48):
                pwu = ps.tile([128, 128], f32, tag="psA", bufs=2,
                              name=f"warm_{wu}")
                nc.tensor.matmul(pwu[:], ident[:], ident[:],
                                 start=True, stop=True)
            cos_sb = pconst.tile([128, S], bf16, tag="cos", bufs=1)
            sin_sb = pconst.tile([128, S], bf16, tag="sin", bufs=1)
            mask_sb = pconst.tile([128, 4, 512], bf16, tag="mask", bufs=1)

            def load_consts():   # emitted after stage A block 0's DMAs
                nc.sync.dma_start(out=cos_sb[:], in_=cosf[:])
                nc.sync.dma_start(out=sin_sb[:], in_=sinm[:])
                nc.sync.dma_start(out=mask_sb[:],
                                  in_=masks[:].rearrange("v p x -> p v x"))
                # zero both score slots once so narrowed diagonal tiles
                # never exp() uninitialized PSUM (reuses see old scores)
                for z in range(2):
                    pss0 = ps.tile([128, 2, 512], f32, tag="pss", bufs=2,
                                   name=f"pss_init_{z}")
                    nc.vector.memset(pss0[:], 0.0)

            # pass-1 k never round-trips through DRAM: stage A blocks
            # 0,1,4,5 write k straight into these persistent tiles, which
            # are roped in place and stay resident for pass 2 (q still
            # spills to DRAM; it dies after pass 1 but all 8 instances'
            # q exist before pass 1 starts, which SBUF can't hold)
            kqp = [pb.tile([128, TH], bf16, tag="kqp", bufs=NI,
                           name=f"kqp_{j}") for j in range(NI)]

            # ---------------- stage A: fused QKV projection ----------------
            # weights are resident in SBUF for the whole of stage A (their
            # tags are recycled by stage C for the w_o stream afterwards);
            # loads are split across DMA rings so block 0 starts fast
            def alloc_weights():
                wv_a = pa.tile([128, NIB2, JCC], bf16, tag="wv_a", bufs=1,
                               name="wv_a")
                wv_b = pa.tile([128, NIB2, JCC], bf16, tag="wv_b", bufs=1,
                               name="wv_b")
                w8a = pa.tile([128, 4 * NIB2, 256], fp8, tag="w8a", bufs=1,
                              name="w8a")
                w8b = pa.tile([128, 4 * NIB2, 256], fp8, tag="w8b", bufs=1,
                              name="w8b")
                return wv_a, wv_b, w8a, w8b

            def stage_a_dma(tb, wts, split=False):
                wv_a, wv_b, w8a, w8b = wts
                n0 = tb * NIB
                hs16a = pa.tile([128, NIB2, TB], bf16, tag="hs16a", bufs=1,
                                name=f"hs16a_{tb}")
                hs16b = pa.tile([128, NIB2, TB], bf16, tag="hs16b", bufs=1,
                                name=f"hs16b_{tb}")
                if split:
                    # block 0: interleave the loads the first v-chain
                    # matmuls need (hs16a + wv_a) across rings first, then
                    # the b-halves, then the kq-phase tensors
                    for sp in range(4):
                        nc.sync.dma_start(
                            out=hs16a[:, 4 * sp:4 * (sp + 1), :],
                            in_=hs16d[:][:, n0 + 4 * sp:n0 + 4 * (sp + 1), :])
                        nc.sync.dma_start(
                            out=wv_a[:, 4 * sp:4 * (sp + 1), :],
                            in_=wTv_v[:, 4 * sp:4 * (sp + 1), :])
                    for sp in range(4):
                        nc.sync.dma_start(
                            out=hs16b[:, 4 * sp:4 * (sp + 1), :],
                            in_=hs16d[:][:, n0 + NIB2 + 4 * sp:
                                         n0 + NIB2 + 4 * (sp + 1), :])
                        nc.sync.dma_start(
                            out=wv_b[:, 4 * sp:4 * (sp + 1), :],
                            in_=wTv_v[:, NIB2 + 4 * sp:
                                      NIB2 + 4 * (sp + 1), :])
                else:
                    nc.sync.dma_start(out=hs16a[:],
                                      in_=hs16d[:][:, n0:n0 + NIB2, :])
                    nc.sync.dma_start(out=hs16b[:],
                                      in_=hs16d[:][:, n0 + NIB2:n0 + NIB, :])
                hs8 = pa.tile([128, NIB, TB], fp8, tag="hs8", bufs=1,
                              name=f"hs8_{tb}")
                nc.sync.dma_start(out=hs8[:],
                                  in_=hs8d[:][:, n0:n0 + NIB, :])
                if split:
                    for sp in range(2):
                        nc.sync.dma_start(
                            out=w8a[:, 32 * sp:32 * (sp + 1), :],
                            in_=wT8[:][:, 32 * sp:32 * (sp + 1), :])
                        nc.sync.dma_start(
                            out=w8b[:, 32 * sp:32 * (sp + 1), :],
                            in_=wT8[:][:, 64 + 32 * sp:64 + 32 * (sp + 1), :])
                return hs16a, hs16b, hs8

            def stage_a_chunks(tb, wts, hs, quantum=4):
                """Generator: emits the block's matmuls in `quantum`-sized
                chunks, yielding 'v' after v-phase chunks and 'kq' after
                kq chunks, so a filler can weave them into attention
                stalls.  v phase first: hs16 dies early."""
                wv_a, wv_b, w8a, w8b = wts
                hs16a, hs16b, hs8 = hs
                u0 = tb * TB
                b = u0 // S
                t0 = u0 % S
                for tt in range(TB // 128):
                    psa = ps.tile([128, JCC], f32, tag="psA", bufs=2,
                                  name=f"psA_v_{tb}_{tt}")
                    for ib0 in range(0, NIB, quantum):
                        for ib in range(ib0, ib0 + quantum):
                            hsrc = hs16a if ib < NIB2 else hs16b
                            wsrc = wv_a if ib < NIB2 else wv_b
                            nc.tensor.matmul(
                                psa[:],
                                hsrc[:, ib % NIB2, tt * 128:(tt + 1) * 128],
                                wsrc[:, ib % NIB2, :],
                                start=(ib == 0), stop=(ib == NIB - 1))
                        if ib0 + quantum < NIB:
                            yield 'v'
                    st = pa.tile([128, JCC], bf16, tag="oA", bufs=2,
                                 name=f"stA_{tb}_v_{tt}")
                    if tt % 2 == 0:
                        nc.scalar.copy(st[:], psa[:])
                    else:
                        nc.vector.tensor_copy(st[:], psa[:])
                    nc.sync.dma_start(
                        out=v_d[b][:][t0 + tt * 128:t0 + (tt + 1) * 128, :],
                        in_=st[:])
                    yield 'v_cb'
                for hp in range(HPC // 2):
                    w8 = w8a if hp == 0 else w8b
                    for d in range(2):
                        hh = 2 * hp + d
                        i = b * HPC + hh
                        for part in range(2):
                            lob = 2 * d + part
                            psb = ps.tile([128, TB], f32, tag="psA",
                                          bufs=2,
                                          name=f"psA_kq_{tb}_{hh}_{part}")
                            for ii0 in range(0, NIB2, quantum):
                                for ii in range(ii0, ii0 + quantum):
                                    nc.tensor.matmul(
                                        psb[:],
                                        w8[:, lob * NIB2 + ii, :],
                                        hs8[:, 2 * ii:2 * ii + 2, :],
                                        start=(ii == 0),
                                        stop=(ii == NIB2 - 1),
                                        perf_mode=DR)
                                if ii0 + quantum < NIB2:
                                    yield 'kq'
                            if tb in (0, 1, 4, 5) and part == 0:
                                # pass-1 k tokens: straight into SBUF
                                nc.vector.tensor_copy(
                                    kqp[i][:, t0:t0 + TB], psb[:])
                            else:
                                st2 = pa.tile([128, TB], bf16, tag="oA",
                                              bufs=2,
                                              name=f"stA_{tb}_{hh}_{part}")
                                if part == 0:
                                    nc.vector.tensor_copy(st2[:], psb[:])
                                else:
                                    nc.scalar.copy(st2[:], psb[:])
                                nc.sync.dma_start(
                                    out=kq_d[2 * i + part][:][:,
                                                              t0:t0 + TB],
                                    in_=st2[:])
                            yield 'kq_cb'

            def stage_a(tb, wts, split=False):
                hs = stage_a_dma(tb, wts, split)
                for _ in stage_a_chunks(tb, wts, hs):
                    pass

            class Filler:
                """Weaves stage-A blocks into attention stalls.  A block's
                DMAs are issued `lead` fill-points before its matmuls are
                emitted (so the data lands first and a fill matmul never
                parks the in-order PE queue); the NEXT block's DMAs go out
                as soon as the current block's v phase is emitted (its
                hs16 buffers are then provably free, so the descriptors
                never park at a ring head)."""

                def __init__(self, blocks, wts, lead=12):
                    self.queue = list(blocks)
                    self.wts = wts
                    self.lead = lead
                    self.gen = None
                    self.skip = 0
                    self.staged = None
                    self.seen_kq = False
                    self.mid_chain = False

                def _issue_next_dma(self):
                    if self.staged is None and self.queue:
                        tb = self.queue.pop(0)
                        self.staged = (tb, stage_a_dma(tb, self.wts))

                def step(self):
                    if self.skip > 0:
                        self.skip -= 1
                        return
                    if self.gen is None:
                        if self.staged is None:
                            if not self.queue:
                                return
                            self._issue_next_dma()
                            self.skip = self.lead
                            return
                        tb, hs = self.staged
                        self.staged = None
                        self.gen = stage_a_chunks(tb, self.wts, hs)
                        self.seen_kq = False
                    try:
                        tag = next(self.gen)
                        self.mid_chain = not tag.endswith('_cb')
                        if tag.startswith('kq') and not self.seen_kq:
                            self.seen_kq = True
                            self._issue_next_dma()
                    except StopIteration:
                        self.gen = None
                        self.mid_chain = False
                        self.step()

                def finish_chain(self):
                    # complete the current psA accumulation chain so a
                    # tail's pden alloc never lands mid-chain (the PE's
                    # in-order queue would deadlock on the rotation)
                    while self.gen is not None and self.mid_chain:
                        self.skip = 0
                        self.step()

                def drain(self):
                    while (self.gen is not None or self.staged is not None
                           or self.queue):
                        self.skip = 0
                        self.step()

            FILL = [None]

            def fill_step():
                if FILL[0] is not None:
                    FILL[0].step()

            # ------------- stage B helpers -------------
            def load_rope(jt, c0, c1, tag, nm, bufs=2):
                X = c1 - c0
                cm = (c0 + c1) // 2
                raw = pb.tile([128, X], bf16, tag="raw", bufs=5,
                              name=f"{nm}_raw")
                nc.sync.dma_start(out=raw[:, 0:X // 2],
                                  in_=kq_d[jt][:][:, c0:cm])
                nc.sync.dma_start(out=raw[:, X // 2:X],
                                  in_=kq_d[jt][:][:, cm:c1])
                sw = pb.tile([128, X], bf16, tag="raw", bufs=5,
                             name=f"{nm}_sw")
                nc.sync.dma_start(out=sw[0:64, :],
                                  in_=kq_d[jt][:][64:128, c0:c1])
                nc.sync.dma_start(out=sw[64:128, :],
                                  in_=kq_d[jt][:][0:64, c0:c1])
                t2 = pb.tile([128, X], bf16, tag="ropetmp", bufs=2,
                             name=f"{nm}_t2")
                rt = pb.tile([128, X], bf16, tag=tag, bufs=bufs,
                             name=f"{nm}_roped")
                with tc.high_priority():
                    nc.vector.tensor_mul(t2[:], sw[:], sin_sb[:, c0:c1])
                    nc.vector.tensor_mul(rt[:], raw[:], cos_sb[:, c0:c1])
                    nc.vector.tensor_add(rt[:], rt[:], t2[:])
                return rt

            def attn_block(i, g, kTs, qT, qoff, v_sb, a2a_t, half,
                           tail_prev):
                """causal attention for q block g (512 q), k blocks 0..4g+3

                Software-pipelined: scores for batch bt are emitted before
                attn@v of batch bt-1; the softmax denominator accumulates
                on the vector engine (f32) and is partition-summed by one
                ones-matmul in the returned tail closure.  tail_prev (the
                previous block's tail) is emitted after this block's first
                score batch so its ones-matmul never stalls the PE.
                """
                b, hh = divmod(i, HPC)
                nbat = g + 1
                nb2 = 2 * nbat           # 2-kb score batches, pipelined
                po = ps.tile([128, 512], f32, tag="po", bufs=2,
                             name=f"po_{half}_{i}_{g}")
                dacc = pb.tile([128, 2, 512], bf16, tag="dacc", bufs=2,
                               name=f"dacc_{half}_{i}_{g}")
                pts = {}

                def scores(bt):
                    diag = (bt >= nb2 - 2)
                    v0 = 2 * bt - (4 * nbat - 4) if diag else 0
                    # whole-batch narrowing: both j tiles of the last
                    # diagonal batch live in [oq:512]; exp/mask/den skip
                    # the dead region (it never reaches po or the den)
                    oq = 128 * v0 if diag else 0
                    pss = ps.tile([128, 2, 512], f32, tag="pss", bufs=2,
                                  name=f"pss_{half}_{i}_{g}_{bt}")
                    for j in range(2):
                        kb = 2 * bt + j
                        off = 128 * (v0 + j) if diag else 0
                        kt, kbl = ((kTs[0], kb) if kb < 8
                                   else (kTs[1], kb - 8))
                        nc.tensor.matmul(
                            pss[:, j, off:512],
                            kt[:, kbl * 128:(kbl + 1) * 128],
                            qT[:, qoff + off:qoff + 512],
                            start=True, stop=True)
                    fill_step()
                    pt = pb.tile([128, 2, 512], bf16, tag="pt", bufs=3,
                                 name=f"pt_{half}_{i}_{g}_{bt}")
                    nc.scalar.activation(pt[:, :, oq:512], pss[:, :, oq:512],
                                         AF.Exp, scale=SCALE)
                    if diag:
                        nc.vector.tensor_mul(pt[:, :, oq:512],
                                             pt[:, :, oq:512],
                                             mask_sb[:, v0:v0 + 2, oq:512])
                    if bt == 0:
                        nc.vector.tensor_copy(dacc[:], pt[:])
                    else:
                        nc.vector.tensor_add(dacc[:, :, oq:512],
                                             dacc[:, :, oq:512],
                                             pt[:, :, oq:512])
                    pts[bt] = pt

                def po_mm(bt):
                    fill_step()
                    diag = (bt >= nb2 - 2)
                    v0 = 2 * bt - (4 * nbat - 4) if diag else 0
                    pt = pts.pop(bt)
                    for j in range(2):
                        kb = 2 * bt + j
                        off = 128 * (v0 + j) if diag else 0
                        nc.tensor.matmul(po[:, off:512], v_sb[:, kb, :],
                                         pt[:, j, off:512],
                                         start=(kb == 0),
                                         stop=(kb == 4 * nbat - 1))

                # two-deep software pipeline: attn@v for batch bt runs two
                # score batches behind, hiding the exp->mask chain latency
                scores(0)
                if tail_prev is not None:
                    tail_prev()
                scores(1)
                for bt in range(2, nb2):
                    scores(bt)
                    po_mm(bt - 2)
                po_mm(nb2 - 2)
                po_mm(nb2 - 1)

                def tail():
                    if FILL[0] is not None:
                        FILL[0].finish_chain()
                    dsum = pb.tile([128, 512], bf16, tag="dsum", bufs=2,
                                   name=f"dsum_{half}_{i}_{g}")
                    nc.vector.tensor_add(dsum[:], dacc[:, 0, :],
                                         dacc[:, 1, :])
                    pden = ps.tile([128, 512], f32, tag="psA", bufs=2,
                                   name=f"pden_{half}_{i}_{g}")
                    nc.tensor.matmul(pden[:], ones16[:], dsum[:],
                                     start=True, stop=True)
                    rden = pb.tile([128, 512], f32, tag="rden", bufs=1,
                                   name=f"rden_{half}_{i}_{g}")
                    nc.vector.reciprocal_approx_fast(out=rden[:], in_=pden[:])
                    attn = pb.tile([128, 512], bf16, tag="attn", bufs=2,
                                   name=f"attn_{half}_{i}_{g}")
                    nc.vector.tensor_mul(attn[:], po[:], rden[:])
                    gl = g - 2 * half    # quarter-pair index within the half
                    for dq in range(2):
                        shard = b * (NCORES // B) + 2 * gl + dq
                        nc.sync.dma_start(
                            out=a2a_t[:][shard, hh * 128:(hh + 1) * 128, :],
                            in_=attn[:, dq * QT:(dq + 1) * QT])
                return tail

            def load_v(i, nkb, half):
                b, hh = divmod(i, HPC)
                v_sb = pb.tile([128, nkb, 128], bf16, tag="vsb", bufs=2,
                               name=f"v_{half}_{i}")
                nc.sync.dma_start(
                    out=v_sb[:, 0:nkb // 2, :],
                    in_=v_d_v[b][:, 0:nkb // 2, hh * 128:(hh + 1) * 128])
                nc.sync.dma_start(
                    out=v_sb[:, nkb // 2:nkb, :],
                    in_=v_d_v[b][:, nkb // 2:nkb, hh * 128:(hh + 1) * 128])
                return v_sb

            # ---------------- emit ----------------
            # stage A blocks: batch0 tokens [0,1024), batch1 [0,1024)
            # pass-1 roped k tiles persist in SBUF so pass 2 only ropes
            # the second token half (saves DVE work + ring traffic)
            kp_tiles = {}

            def rope_inplace(jt, nm):
                t = kqp[jt]
                sw = pb.tile([128, TH], bf16, tag="raw", bufs=5,
                             name=f"{nm}_sw")
                nc.sync.dma_start(out=sw[0:64, :], in_=t[64:128, :])
                nc.sync.dma_start(out=sw[64:128, :], in_=t[0:64, :])
                t2 = pb.tile([128, TH], bf16, tag="ropetmp", bufs=2,
                             name=f"{nm}_t2")
                with tc.high_priority():
                    nc.vector.tensor_mul(t2[:], sw[:], sin_sb[:, 0:TH])
                    nc.vector.tensor_mul(t[:], t[:], cos_sb[:, 0:TH])
                    nc.vector.tensor_add(t[:], t[:], t2[:])
                return t

            def pass1_inst(i, tail):
                kT = rope_inplace(i, f"k1_{i}")
                kp_tiles[i] = kT
                qT = load_rope(2 * i + 1, 0, TH, "qr_r", f"q1_{i}")
                v_sb = load_v(i, TH // 128, 0)
                for g in range(2):
                    tail = attn_block(i, g, (kT, None), qT, g * 512, v_sb,
                                      a2a_in[0], 0, tail)
                return tail

            def pass2_loads(i):
                kT2 = load_rope(2 * i, TH, S, "kr_r", f"k2_{i}")
                qT = load_rope(2 * i + 1, TH, S, "qr_r", f"q2_{i}")
                v_sb = load_v(i, S // 128, 1)
                return kT2, qT, v_sb

            def pass2_inst(i, tail, pre=None):
                kT2, qT, v_sb = pre if pre is not None else pass2_loads(i)
                for g in range(2, 4):
                    tail = attn_block(i, g, (kp_tiles[i], kT2), qT,
                                      (g - 2) * 512, v_sb,
                                      a2a_in[1], 1, tail)
                return tail

            at_tags = ["at0", "hs16a"]

            def c_at_alloc(half):
                return pa.tile([128, NJB, QT], bf16, tag=at_tags[half],
                               bufs=1, name=f"at_{half}")

            def c_at_dma(at, half, chunks, eng):
                # the Activation DGE queue is slow (~23 GB/s serial) but
                # idle, so descriptors parked on an unmet dependency block
                # nothing; sync-ring chunks are only emitted once their
                # dependency is certainly met
                for sp in chunks:
                    eng.dma_start(out=at[:, 8 * sp:8 * (sp + 1), :],
                                  in_=at_v[half][:, 8 * sp:8 * (sp + 1), :])

            # wo stream recycles the stage-A weight tags; the jb<16 / jb>=16
            # split means each slot frees at mid-block, giving half a block
            # of load-ahead even with single-buffered tags
            wo_tags = ["wv_a", "wv_b", "w8a", "w8b"]

            def c_wo_load(half, mb):
                tagA = wo_tags[2 * (mb % 2)]
                tagB = wo_tags[2 * (mb % 2) + 1]
                woA = pa.tile([128, 16, 512], bf16, tag=tagA, bufs=1,
                              name=f"woA_{half}_{mb}")
                woB = pa.tile([128, 16, 512], bf16, tag=tagB, bufs=1,
                              name=f"woB_{half}_{mb}")
                nc.sync.dma_start(
                    out=woA[:], in_=woT_v[:, 0:16, mb * 512:(mb + 1) * 512])
                nc.sync.dma_start(
                    out=woB[:], in_=woT_v[:, 16:32, mb * 512:(mb + 1) * 512])
                return woA, woB

            def c_chunks(half, mb, at, woA, woB, quantum=4):
                for t in range(QT // 128):
                    psc = ps.tile([128, 512], f32, tag="psA", bufs=2,
                                  name=f"psC_{half}_{mb}_{t}")
                    for jb0 in range(0, NJB, quantum):
                        for jb in range(jb0, jb0 + quantum):
                            wsrc = woA if jb < 16 else woB
                            nc.tensor.matmul(
                                psc[:],
                                at[:, jb, t * 128:(t + 1) * 128],
                                wsrc[:, jb % 16, :],
                                start=(jb == 0), stop=(jb == NJB - 1))
                        if jb0 + quantum < NJB:
                            yield 'c'
                    oc = pc.tile([128, 512], bf16, tag="oC", bufs=2,
                                 name=f"oC_{half}_{mb}_{t}")
                    nc.scalar.copy(oc[:], psc[:])
                    r0 = half * QT + t * 128
                    for sp in range(2):
                        nc.sync.dma_start(
                            out=out[:][r0:r0 + 128,
                                       mb * 512 + 256 * sp:
                                       mb * 512 + 256 * (sp + 1)],
                            in_=oc[:, 256 * sp:256 * (sp + 1)])
                    yield 'c_cb'

            def c_compute(half, mb, at, woA, woB):
                for _ in c_chunks(half, mb, at, woA, woB):
                    pass

            with nc.named_scope("stageA01"):
                wts = alloc_weights()
                stage_a(0, wts, split=True)
                load_consts()
                for tb in (1, 4, 5):
                    stage_a(tb, wts)
            with nc.named_scope("pass1"):
                # stage-A blocks 2,3 are woven into pass 1's attention
                # stalls (one 4-matmul chunk per score batch); what's left
                # drains densely after the collective is enqueued
                FILL[0] = filler1 = Filler([2, 3], wts)
                tail = None
                for i in range(NI):
                    tail = pass1_inst(i, tail)
                tail()
                FILL[0] = None
            nc.gpsimd.collective_compute(
                "AllToAll", mybir.AluOpType.bypass, replica_groups=GROUPS,
                ins=[a2a_in[0][:]], outs=[a2a_out[0][:]])

            with nc.named_scope("stageA23"):
                filler1.drain()
                # issue pass 2's first loads right after the drain (their
                # kq/v stores are all emitted by now) so the descriptors
                # precede the A2A #1 data phase on the rings
                pre2 = [pass2_loads(0), pass2_loads(1)]
            with nc.named_scope("pass2"):
                # blocks 6,7 fill pass 2's batch-0 instances and must be
                # fully drained before the batch-1 instances read them
                FILL[0] = filler2 = Filler([6, 7], wts)
                tail = None
                for i in range(NI):
                    if i == 0:
                        # at0 has a dedicated buffer (no WAR dependency),
                        # so its slow Act-queue transfer starts right at
                        # pass-2 start and lands well before stage C
                        at0 = c_at_alloc(0)
                        c_at_dma(at0, 0, [0, 1, 2, 3], nc.scalar)
                    if i == 4:
                        FILL[0] = None
                        filler2.drain()
                    if i == 5:
                        # prefetch stage C half-0 weights while pass 2 runs
                        # (safe: buffer and input deps are already met when
                        # the descriptors reach the DMA rings; a DMA whose
                        # deps resolve only after a collective would block
                        # its ring and starve loads queued behind it).
                        # Two blocks are resident before A2A #2's data
                        # phase hogs the rings.
                        wo00 = c_wo_load(0, 0)
                    if i == 6:
                        wo01 = c_wo_load(0, 1)
                    tail = pass2_inst(i, tail,
                                      pre2[i] if i < len(pre2) else None)
                tail()
            nc.gpsimd.collective_compute(
                "AllToAll", mybir.AluOpType.bypass, replica_groups=GROUPS,
                ins=[a2a_in[1][:]], outs=[a2a_out[1][:]])

            # ---------------- stage C: token-quarter o_proj ----------------
            with nc.named_scope("stageC"):
                at1 = c_at_alloc(1)
                # first half parks on the idle Act queue until A2A #2
                c_at_dma(at1, 1, [0, 1], nc.scalar)
                for mb in range(H // 512):
                    if mb == 0:
                        woA, woB = wo00
                    elif mb == 1:
                        woA, woB = wo01
                    else:
                        woA, woB = c_wo_load(0, mb)
                    c_compute(0, mb, at0, woA, woB)
                # by now A2A #2 has certainly landed: the rest of at1 can
                # go on the fast sync rings without parking them
                c_at_dma(at1, 1, [2, 3], nc.sync)
                for mb in range(H // 512):
                    woA, woB = c_wo_load(1, mb)
                    c_compute(1, mb, at1, woA, woB)

    nc.finalize()
    return nc


_NC_CACHE = None


def _get_nc():
    global _NC_CACHE
    if _NC_CACHE is None:
        _NC_CACHE = build_nc()
    return _NC_CACHE


def _host_inputs(hidden_states, positions, w_pack, w_o):
    hidden_states = np.asarray(hidden_states, dtype=np.float32)
    positions = np.asarray(positions)
    w_pack = np.asarray(w_pack, dtype=np.float32)
    w_o = np.asarray(w_o, dtype=np.float32)

    half = HD // 2
    inv_freq = (1.0 / (THETA ** (np.arange(half, dtype=np.float32) / half)))

    # causal mask variants for the 4 diagonal (128x512) tiles of a q-block
    masks = np.empty((4, 128, 512), dtype=np.float32)
    xs = np.arange(512)[None, :]
    ps = np.arange(128)[:, None]
    for v in range(4):
        masks[v] = (xs >= ps + 128 * v).astype(np.float32)

    woT_full = np.ascontiguousarray(w_o.T).astype(ml_dtypes.bfloat16)
    # both batches side by side: [H, B*S]
    hsT = np.concatenate([hidden_states[0].T, hidden_states[1].T], axis=1)
    # blocked [128 p, (tb n), t] layout for contiguous stage-A loads
    hsb = np.ascontiguousarray(
        hsT.reshape(NIB, 128, NTB, TB).transpose(1, 2, 0, 3)
        .reshape(128, NTB * NIB, TB))
    hs16d = hsb.astype(ml_dtypes.bfloat16)
    hs8d = (hsb * SCALEQK).astype(ml_dtypes.float8_e4m3)

    ang = positions[0].astype(np.float32)[None, :] * inv_freq[:, None]
    cos_t = np.cos(ang).astype(np.float32)                 # [64, S]
    sin_t = np.sin(ang).astype(np.float32)
    dsc = 1.0 / (SCALEQK * SCALEQK)
    cosf = (np.concatenate([cos_t, cos_t], axis=0) * dsc) \
        .astype(ml_dtypes.bfloat16)
    sinm = (np.concatenate([-sin_t, sin_t], axis=0) * dsc) \
        .astype(ml_dtypes.bfloat16)
    masks16 = masks.astype(ml_dtypes.bfloat16)

    in_maps = []
    for c in range(NCORES):
        heads = np.arange(HPC * c, HPC * (c + 1))
        kq_parts, v_parts = [], []
        for h in heads:
            hr = np.arange(h * HD, (h + 1) * HD)
            kq_parts += [w_pack[H + hr], w_pack[hr]]       # k then q
            v_parts.append(w_pack[2 * H + hr])
        wT_kq = np.concatenate(kq_parts, axis=0).T             # [H, 2*JCC]
        # SwInterleave stationary: [p, ob*16+i, 256] with col 2c+m =
        # member m's weight column (127-c)
        A = wT_kq.reshape(16, 2, 128, 8, 128)[:, :, :, :, ::-1]
        wT8 = (A.transpose(2, 3, 0, 4, 1).reshape(128, 128, 256)
               * SCALEQK)
        wTv = np.concatenate(v_parts, axis=0).T                # [H, JCC]
        in_maps.append({
            "hs16d": hs16d,
            "hs8d": hs8d,
            "wT8": np.ascontiguousarray(wT8).astype(ml_dtypes.float8_e4m3),
            "wTv": np.ascontiguousarray(wTv).astype(ml_dtypes.bfloat16),
            "woT": woT_full,
            "cosf": cosf,
            "sinm": sinm,
            "masks": masks16,
        })
    return in_maps


def _assemble(results):
    out = np.empty((B, S, H), dtype=np.float32)
    for c in range(NCORES):
        b, q = divmod(c, NCORES // B)
        res = results[c]["out"]                    # [2*QT, H]
        out[b][QT * q:QT * (q + 1)] = res[:QT]
        out[b][TH + QT * q:TH + QT * (q + 1)] = res[QT:]
    return out


def kernel(hidden_states, positions, w_pack, w_o):
    import os
    os.environ["BASS_NEVER_TRACE"] = "1"
    nc = _get_nc()
    in_maps = _host_inputs(hidden_states, positions, w_pack, w_o)
    res = run_bass_kernel_spmd(nc, in_maps, list(range(NCORES)))
    return _assemble(res.results)


# revision 74
# speedup vs baseline: 1.0175x; 1.0175x over previous
"""BaiChuan attention layer on 8 Trainium2 NeuronCores.

Sharding: tensor-parallel over heads across all 8 cores (4 heads per
core, both batches on every core).  o_proj is token-parallel: per-head
attention outputs are exchanged with one 8-way AllToAll per causal
token-half, after which each core computes the full o_proj for its
(batch, token-quarter) share with w_o streamed from HBM.

Precision: q/k projection runs in fp8e4 with DoubleRow (inputs scaled by
64, descaled via the rope cos/sin tables which are pre-divided by 4096);
everything else is bf16 operands with f32 PSUM accumulation.  Softmax
denominator accumulates on the vector engine (bf16, 2x mode) and is
partition-summed with one ones-matmul per attention block.  The output
leaves the device in bf16 and is upcast on the host.

Pipeline (core c: heads 4c..4c+4 of both batches; owns output tokens
[256*(c%4), +256) and [1024+256*(c%4), +256) of batch c//4):
  stage A  (8 token blocks of 512 over batch0++batch1): qkvT[j,t];
           v in bf16, q/k in fp8 DoubleRow.  hs inputs are host-packed
           into a blocked layout so every load is one contiguous 16KB
           run per partition; wv/w8 weights are SBUF-resident (loaded
           once, tags recycled by stage C's w_o stream); pass-1 k goes
           straight into persistent SBUF tiles (no DRAM round-trip).
           Only blocks 0,1,4,5 are emitted up front: blocks 2,3 / 6,7
           are woven into pass 1 / pass 2 as 4-matmul fill chunks
           (Filler) so the PE stays dense through the attention phases.
  pass 1   per head-instance: RoPE (k in place, q via DRAM round-trip),
           causal attention for q in [0,1024).  The score->exp->attn@v
           chain is pipelined two score-batches deep so the PE never
           waits on the exp->mask latency.
  A2A #1   8-way AllToAll of all instances' attn[:, 0:1024].
  pass 2   attention for q in [1024,2048) (resident roped k for the
           first token half, only [1024:2048) re-roped).
  A2A #2   second half exchange.
  stage C  o_proj per token quarter: out[t, m] over all 32 heads, w_o
           streamed per 512-wide m block; half-0's at/wo tiles are
           prefetched during pass 2 (collective-dependent loads ride
           the otherwise-idle Act DGE queue so a parked descriptor
           never blocks the SP DMA rings).
"""
import sys
sys.path.insert(0, '/opt/trn_rl_repo')
import numpy as np
import ml_dtypes

import concourse.bass as bass
from concourse import bacc
import concourse.mybir as mybir
from concourse.tile import TileContext
from concourse.bass_utils import run_bass_kernel_spmd
from concourse.masks import make_identity

f32 = mybir.dt.float32
bf16 = mybir.dt.bfloat16
fp8 = mybir.dt.float8e4
AF = mybir.ActivationFunctionType
DR = mybir.MatmulPerfMode.DoubleRowSwInterleave

B, S, H, NH = 2, 2048, 4096, 32
HD = H // NH                    # 128
THETA = 10000.0
NCORES = 8
HPC = NH // NCORES              # 4 heads per core
NI = B * HPC                    # 8 head-instances (batch, head) per core
JCC = HPC * HD                  # 512 = per-core head width (one batch)
SCALE = HD ** -0.5
SCALEQK = 64.0                  # fp8 input scaling for the q/k projection
GROUPS = [[0, 1, 2, 3, 4, 5, 6, 7]]
TB = 512                        # stage-A token block
NTB = B * S // TB               # 8 blocks over batch0 ++ batch1
NIB = H // 128                  # 32 contraction blocks
TH = S // 2                     # causal token half
QT = 256                        # tokens per (core, half)
NIB2 = NIB // 2
NJB = NH                        # 32 o_proj j-blocks


def build_nc():
    nc = bacc.Bacc(None)
    # blocked hs layouts: [128 p, (tb n), t] so stage-A loads are
    # contiguous 16KB runs per partition
    hs16d = nc.declare_dram_parameter("hs16d", [128, NTB * NIB, TB], bf16,
                                      isOutput=False)
    hs8d = nc.declare_dram_parameter("hs8d", [128, NTB * NIB, TB], fp8,
                                     isOutput=False)
    # SwInterleave stationary blocks: [128 p, ob*16+ibpair, 256 interleaved]
    # ob = kq out-block (hh*2 + part) of this core, 8 per core
    wT8 = nc.declare_dram_parameter("wT8", [128, 8 * NIB2, 256], fp8,
                                    isOutput=False)
    wTv = nc.declare_dram_parameter("wTv", [H, JCC], bf16, isOutput=False)
    woT = nc.declare_dram_parameter("woT", [H, H], bf16, isOutput=False)
    cosf = nc.declare_dram_parameter("cosf", [HD, S], bf16, isOutput=False)
    sinm = nc.declare_dram_parameter("sinm", [HD, S], bf16, isOutput=False)
    masks = nc.declare_dram_parameter("masks", [4, 128, 512], bf16,
                                      isOutput=False)
    # output leaves the device in bf16 (host upcasts); halves the stage-C
    # store traffic and the end-of-kernel drain
    out = nc.declare_dram_parameter("out", [2 * QT, H], bf16, isOutput=True)

    # per head-instance i = b*HPC + hh: k/q at kq_d[2i + 0/1]
    kq_d = [nc.dram_tensor(f"kq_d{j}", [128, S], bf16)
            for j in range(2 * NI)]
    # v in natural [t, j] layout, one tensor per batch
    v_d = [nc.dram_tensor(f"v_d{b}", [S, JCC], bf16) for b in range(B)]
    a2a_in = [nc.dram_tensor(f"a2a{i}_in", [NCORES, JCC, QT], bf16)
              for i in range(2)]
    a2a_out = [nc.dram_tensor(f"a2a{i}_out", [NCORES, JCC, QT], bf16)
               for i in range(2)]

    wTv_v = wTv[:].rearrange("(n p) j -> p n j", p=128)       # [128, 32, 512]
    v_d_v = [t[:].rearrange("(kb p) j -> p kb j", p=128)      # [128, 16, 512]
             for t in v_d]
    woT_v = woT[:].rearrange("(n p) m -> p n m", p=128)       # [128, 32, H]
    at_v = [t[:].rearrange("r (n p) t -> p (r n) t", p=128)   # [128, 32, QT]
            for t in a2a_out]

    with TileContext(nc) as tc:
        with tc.tile_pool(name="const", bufs=1) as pconst, \
             tc.tile_pool(name="stA", bufs=1) as pa, \
             tc.tile_pool(name="stB", bufs=1) as pb, \
             tc.tile_pool(name="stC", bufs=1) as pc, \
             tc.tile_pool(name="psum", bufs=1, space="PSUM") as ps:
            # ---------------- constants (PE warmup first) ----------------
            ident = pconst.tile([128, 128], bf16, tag="ident", bufs=1)
            make_identity(nc, ident[:])
            ones16 = pconst.tile([128, 128], bf16, tag="ones16", bufs=1)
            nc.vector.memset(ones16[:], 1.0)
            # PE warmup burst (HAM un-throttle while first DMAs land)
            for wu in range(48):
                pwu = ps.tile([128, 128], f32, tag="psA", bufs=2,
                              name=f"warm_{wu}")
                nc.tensor.matmul(pwu[:], ident[:], ident[:],
                                 start=True, stop=True)
            cos_sb = pconst.tile([128, S], bf16, tag="cos", bufs=1)
            sin_sb = pconst.tile([128, S], bf16, tag="sin", bufs=1)
            mask_sb = pconst.tile([128, 4, 512], bf16, tag="mask", bufs=1)

            def load_consts():   # emitted after stage A block 0's DMAs
                nc.sync.dma_start(out=cos_sb[:], in_=cosf[:])
                nc.sync.dma_start(out=sin_sb[:], in_=sinm[:])
                nc.sync.dma_start(out=mask_sb[:],
                                  in_=masks[:].rearrange("v p x -> p v x"))
                # zero both score slots once so narrowed diagonal tiles
                # never exp() uninitialized PSUM (reuses see old scores)
                for z in range(2):
                    pss0 = ps.tile([128, 2, 512], f32, tag="pss", bufs=2,
                                   name=f"pss_init_{z}")
                    nc.vector.memset(pss0[:], 0.0)

            # pass-1 k never round-trips through DRAM: stage A blocks
            # 0,1,4,5 write k straight into these persistent tiles, which
            # are roped in place and stay resident for pass 2 (q still
            # spills to DRAM; it dies after pass 1 but all 8 instances'
            # q exist before pass 1 starts, which SBUF can't hold)
            kqp = [pb.tile([128, TH], bf16, tag="kqp", bufs=NI,
                           name=f"kqp_{j}") for j in range(NI)]

            # ---------------- stage A: fused QKV projection ----------------
            # weights are resident in SBUF for the whole of stage A (their
            # tags are recycled by stage C for the w_o stream afterwards);
            # loads are split across DMA rings so block 0 starts fast
            def alloc_weights():
                wv_a = pa.tile([128, NIB2, JCC], bf16, tag="wv_a", bufs=1,
                               name="wv_a")
                wv_b = pa.tile([128, NIB2, JCC], bf16, tag="wv_b", bufs=1,
                               name="wv_b")
                w8a = pa.tile([128, 4 * NIB2, 256], fp8, tag="w8a", bufs=1,
                              name="w8a")
                w8b = pa.tile([128, 4 * NIB2, 256], fp8, tag="w8b", bufs=1,
                              name="w8b")
                return wv_a, wv_b, w8a, w8b

            def stage_a_dma(tb, wts, split=False):
                wv_a, wv_b, w8a, w8b = wts
                n0 = tb * NIB
                hs16a = pa.tile([128, NIB2, TB], bf16, tag="hs16a", bufs=1,
                                name=f"hs16a_{tb}")
                hs16b = pa.tile([128, NIB2, TB], bf16, tag="hs16b", bufs=1,
                                name=f"hs16b_{tb}")
                if split:
                    # block 0: interleave the loads the first v-chain
                    # matmuls need (hs16a + wv_a) across rings first, then
                    # the b-halves, then the kq-phase tensors
                    for sp in range(4):
                        nc.sync.dma_start(
                            out=hs16a[:, 4 * sp:4 * (sp + 1), :],
                            in_=hs16d[:][:, n0 + 4 * sp:n0 + 4 * (sp + 1), :])
                        nc.sync.dma_start(
                            out=wv_a[:, 4 * sp:4 * (sp + 1), :],
                            in_=wTv_v[:, 4 * sp:4 * (sp + 1), :])
                    for sp in range(4):
                        nc.sync.dma_start(
                            out=hs16b[:, 4 * sp:4 * (sp + 1), :],
                            in_=hs16d[:][:, n0 + NIB2 + 4 * sp:
                                         n0 + NIB2 + 4 * (sp + 1), :])
                        nc.sync.dma_start(
                            out=wv_b[:, 4 * sp:4 * (sp + 1), :],
                            in_=wTv_v[:, NIB2 + 4 * sp:
                                      NIB2 + 4 * (sp + 1), :])
                else:
                    # 2-way splits: the block lands on twice the rings,
                    # halving the window in which fills can't run
                    for sp in range(2):
                        nc.sync.dma_start(
                            out=hs16a[:, 8 * sp:8 * (sp + 1), :],
                            in_=hs16d[:][:, n0 + 8 * sp:n0 + 8 * (sp + 1),
                                         :])
                        nc.sync.dma_start(
                            out=hs16b[:, 8 * sp:8 * (sp + 1), :],
                            in_=hs16d[:][:, n0 + NIB2 + 8 * sp:
                                         n0 + NIB2 + 8 * (sp + 1), :])
                hs8 = pa.tile([128, NIB, TB], fp8, tag="hs8", bufs=1,
                              name=f"hs8_{tb}")
                for sp in range(2):
                    nc.sync.dma_start(
                        out=hs8[:, 16 * sp:16 * (sp + 1), :],
                        in_=hs8d[:][:, n0 + 16 * sp:n0 + 16 * (sp + 1), :])
                if split:
                    for sp in range(2):
                        nc.sync.dma_start(
                            out=w8a[:, 32 * sp:32 * (sp + 1), :],
                            in_=wT8[:][:, 32 * sp:32 * (sp + 1), :])
                        nc.sync.dma_start(
                            out=w8b[:, 32 * sp:32 * (sp + 1), :],
                            in_=wT8[:][:, 64 + 32 * sp:64 + 32 * (sp + 1), :])
                return hs16a, hs16b, hs8

            def stage_a_chunks(tb, wts, hs, quantum=4):
                """Generator: emits the block's matmuls in `quantum`-sized
                chunks, yielding 'v' after v-phase chunks and 'kq' after
                kq chunks, so a filler can weave them into attention
                stalls.  v phase first: hs16 dies early."""
                wv_a, wv_b, w8a, w8b = wts
                hs16a, hs16b, hs8 = hs
                u0 = tb * TB
                b = u0 // S
                t0 = u0 % S
                for tt in range(TB // 128):
                    psa = ps.tile([128, JCC], f32, tag="psA", bufs=2,
                                  name=f"psA_v_{tb}_{tt}")
                    for ib0 in range(0, NIB, quantum):
                        for ib in range(ib0, ib0 + quantum):
                            hsrc = hs16a if ib < NIB2 else hs16b
                            wsrc = wv_a if ib < NIB2 else wv_b
                            nc.tensor.matmul(
                                psa[:],
                                hsrc[:, ib % NIB2, tt * 128:(tt + 1) * 128],
                                wsrc[:, ib % NIB2, :],
                                start=(ib == 0), stop=(ib == NIB - 1))
                        if ib0 + quantum < NIB:
                            yield 'v'
                    st = pa.tile([128, JCC], bf16, tag="oA", bufs=2,
                                 name=f"stA_{tb}_v_{tt}")
                    if tt % 2 == 0:
                        nc.scalar.copy(st[:], psa[:])
                    else:
                        nc.vector.tensor_copy(st[:], psa[:])
                    nc.sync.dma_start(
                        out=v_d[b][:][t0 + tt * 128:t0 + (tt + 1) * 128, :],
                        in_=st[:])
                    yield 'v_cb'
                for hp in range(HPC // 2):
                    w8 = w8a if hp == 0 else w8b
                    for d in range(2):
                        hh = 2 * hp + d
                        i = b * HPC + hh
                        for part in range(2):
                            lob = 2 * d + part
                            psb = ps.tile([128, TB], f32, tag="psA",
                                          bufs=2,
                                          name=f"psA_kq_{tb}_{hh}_{part}")
                            for ii0 in range(0, NIB2, quantum):
                                for ii in range(ii0, ii0 + quantum):
                                    nc.tensor.matmul(
                                        psb[:],
                                        w8[:, lob * NIB2 + ii, :],
                                        hs8[:, 2 * ii:2 * ii + 2, :],
                                        start=(ii == 0),
                                        stop=(ii == NIB2 - 1),
                                        perf_mode=DR)
                                if ii0 + quantum < NIB2:
                                    yield 'kq'
                            if tb in (0, 1, 4, 5) and part == 0:
                                # pass-1 k tokens: straight into SBUF
                                nc.vector.tensor_copy(
                                    kqp[i][:, t0:t0 + TB], psb[:])
                            else:
                                st2 = pa.tile([128, TB], bf16, tag="oA",
                                              bufs=2,
                                              name=f"stA_{tb}_{hh}_{part}")
                                if part == 0:
                                    nc.vector.tensor_copy(st2[:], psb[:])
                                else:
                                    nc.scalar.copy(st2[:], psb[:])
                                nc.sync.dma_start(
                                    out=kq_d[2 * i + part][:][:,
                                                              t0:t0 + TB],
                                    in_=st2[:])
                            yield 'kq_cb'

            def stage_a(tb, wts, split=False):
                hs = stage_a_dma(tb, wts, split)
                for _ in stage_a_chunks(tb, wts, hs):
                    pass

            class Filler:
                """Weaves stage-A blocks into attention stalls.  A block's
                DMAs are issued `lead` fill-points before its matmuls are
                emitted (so the data lands first and a fill matmul never
                parks the in-order PE queue); the NEXT block's DMAs go out
                as soon as the current block's v phase is emitted (its
                hs16 buffers are then provably free, so the descriptors
                never park at a ring head)."""

                def __init__(self, blocks, wts, lead=8):
                    self.queue = list(blocks)
                    self.wts = wts
                    self.lead = lead
                    self.gen = None
                    self.skip = 0
                    self.staged = None
                    self.seen_kq = False
                    self.mid_chain = False

                def _issue_next_dma(self):
                    if self.staged is None and self.queue:
                        tb = self.queue.pop(0)
                        self.staged = (tb, stage_a_dma(tb, self.wts))

                def step(self):
                    if self.skip > 0:
                        self.skip -= 1
                        return
                    if self.gen is None:
                        if self.staged is None:
                            if not self.queue:
                                return
                            self._issue_next_dma()
                            self.skip = self.lead
                            return
                        tb, hs = self.staged
                        self.staged = None
                        self.gen = stage_a_chunks(tb, self.wts, hs)
                        self.seen_kq = False
                    try:
                        tag = next(self.gen)
                        self.mid_chain = not tag.endswith('_cb')
                        if tag.startswith('kq') and not self.seen_kq:
                            self.seen_kq = True
                            self._issue_next_dma()
                    except StopIteration:
                        self.gen = None
                        self.mid_chain = False
                        self.step()

                def finish_chain(self):
                    # complete the current psA accumulation chain so a
                    # tail's pden alloc never lands mid-chain (the PE's
                    # in-order queue would deadlock on the rotation)
                    while self.gen is not None and self.mid_chain:
                        self.skip = 0
                        self.step()

                def drain(self):
                    while (self.gen is not None or self.staged is not None
                           or self.queue):
                        self.skip = 0
                        self.step()

            FILL = [None]

            def fill_step():
                if FILL[0] is not None:
                    FILL[0].step()

            # ------------- stage B helpers -------------
            def load_rope(jt, c0, c1, tag, nm, bufs=2):
                X = c1 - c0
                cm = (c0 + c1) // 2
                raw = pb.tile([128, X], bf16, tag="raw", bufs=5,
                              name=f"{nm}_raw")
                nc.sync.dma_start(out=raw[:, 0:X // 2],
                                  in_=kq_d[jt][:][:, c0:cm])
                nc.sync.dma_start(out=raw[:, X // 2:X],
                                  in_=kq_d[jt][:][:, cm:c1])
                sw = pb.tile([128, X], bf16, tag="raw", bufs=5,
                             name=f"{nm}_sw")
                nc.sync.dma_start(out=sw[0:64, :],
                                  in_=kq_d[jt][:][64:128, c0:c1])
                nc.sync.dma_start(out=sw[64:128, :],
                                  in_=kq_d[jt][:][0:64, c0:c1])
                t2 = pb.tile([128, X], bf16, tag="ropetmp", bufs=2,
                             name=f"{nm}_t2")
                rt = pb.tile([128, X], bf16, tag=tag, bufs=bufs,
                             name=f"{nm}_roped")
                with tc.high_priority():
                    nc.vector.tensor_mul(t2[:], sw[:], sin_sb[:, c0:c1])
                    nc.vector.tensor_mul(rt[:], raw[:], cos_sb[:, c0:c1])
                    nc.vector.tensor_add(rt[:], rt[:], t2[:])
                return rt

            def attn_block(i, g, kTs, qT, qoff, v_sb, a2a_t, half,
                           tail_prev):
                """causal attention for q block g (512 q), k blocks 0..4g+3

                Software-pipelined: scores for batch bt are emitted before
                attn@v of batch bt-1; the softmax denominator accumulates
                on the vector engine (f32) and is partition-summed by one
                ones-matmul in the returned tail closure.  tail_prev (the
                previous block's tail) is emitted after this block's first
                score batch so its ones-matmul never stalls the PE.
                """
                b, hh = divmod(i, HPC)
                nbat = g + 1
                nb2 = 2 * nbat           # 2-kb score batches, pipelined
                po = ps.tile([128, 512], f32, tag="po", bufs=2,
                             name=f"po_{half}_{i}_{g}")
                dacc = pb.tile([128, 2, 512], bf16, tag="dacc", bufs=2,
                               name=f"dacc_{half}_{i}_{g}")
                pts = {}

                def scores(bt):
                    diag = (bt >= nb2 - 2)
                    v0 = 2 * bt - (4 * nbat - 4) if diag else 0
                    # whole-batch narrowing: both j tiles of the last
                    # diagonal batch live in [oq:512]; exp/mask/den skip
                    # the dead region (it never reaches po or the den)
                    oq = 128 * v0 if diag else 0
                    pss = ps.tile([128, 2, 512], f32, tag="pss", bufs=2,
                                  name=f"pss_{half}_{i}_{g}_{bt}")
                    for j in range(2):
                        kb = 2 * bt + j
                        off = 128 * (v0 + j) if diag else 0
                        kt, kbl = ((kTs[0], kb) if kb < 8
                                   else (kTs[1], kb - 8))
                        nc.tensor.matmul(
                            pss[:, j, off:512],
                            kt[:, kbl * 128:(kbl + 1) * 128],
                            qT[:, qoff + off:qoff + 512],
                            start=True, stop=True)
                    fill_step()
                    pt = pb.tile([128, 2, 512], bf16, tag="pt", bufs=3,
                                 name=f"pt_{half}_{i}_{g}_{bt}")
                    nc.scalar.activation(pt[:, :, oq:512], pss[:, :, oq:512],
                                         AF.Exp, scale=SCALE)
                    if diag:
                        nc.vector.tensor_mul(pt[:, :, oq:512],
                                             pt[:, :, oq:512],
                                             mask_sb[:, v0:v0 + 2, oq:512])
                    if bt == 0:
                        nc.vector.tensor_copy(dacc[:], pt[:])
                    else:
                        nc.vector.tensor_add(dacc[:, :, oq:512],
                                             dacc[:, :, oq:512],
                                             pt[:, :, oq:512])
                    pts[bt] = pt

                def po_mm(bt):
                    fill_step()
                    diag = (bt >= nb2 - 2)
                    v0 = 2 * bt - (4 * nbat - 4) if diag else 0
                    pt = pts.pop(bt)
                    for j in range(2):
                        kb = 2 * bt + j
                        off = 128 * (v0 + j) if diag else 0
                        nc.tensor.matmul(po[:, off:512], v_sb[:, kb, :],
                                         pt[:, j, off:512],
                                         start=(kb == 0),
                                         stop=(kb == 4 * nbat - 1))

                # two-deep software pipeline: attn@v for batch bt runs two
                # score batches behind, hiding the exp->mask chain latency
                scores(0)
                if tail_prev is not None:
                    tail_prev()
                scores(1)
                for bt in range(2, nb2):
                    scores(bt)
                    po_mm(bt - 2)
                po_mm(nb2 - 2)
                po_mm(nb2 - 1)

                def tail():
                    if FILL[0] is not None:
                        FILL[0].finish_chain()
                    dsum = pb.tile([128, 512], bf16, tag="dsum", bufs=2,
                                   name=f"dsum_{half}_{i}_{g}")
                    nc.vector.tensor_add(dsum[:], dacc[:, 0, :],
                                         dacc[:, 1, :])
                    pden = ps.tile([128, 512], f32, tag="psA", bufs=2,
                                   name=f"pden_{half}_{i}_{g}")
                    nc.tensor.matmul(pden[:], ones16[:], dsum[:],
                                     start=True, stop=True)
                    rden = pb.tile([128, 512], f32, tag="rden", bufs=1,
                                   name=f"rden_{half}_{i}_{g}")
                    nc.vector.reciprocal_approx_fast(out=rden[:], in_=pden[:])
                    attn = pb.tile([128, 512], bf16, tag="attn", bufs=2,
                                   name=f"attn_{half}_{i}_{g}")
                    nc.vector.tensor_mul(attn[:], po[:], rden[:])
                    gl = g - 2 * half    # quarter-pair index within the half
                    for dq in range(2):
                        shard = b * (NCORES // B) + 2 * gl + dq
                        nc.sync.dma_start(
                            out=a2a_t[:][shard, hh * 128:(hh + 1) * 128, :],
                            in_=attn[:, dq * QT:(dq + 1) * QT])
                return tail

            def load_v(i, nkb, half):
                b, hh = divmod(i, HPC)
                v_sb = pb.tile([128, nkb, 128], bf16, tag="vsb", bufs=2,
                               name=f"v_{half}_{i}")
                nc.sync.dma_start(
                    out=v_sb[:, 0:nkb // 2, :],
                    in_=v_d_v[b][:, 0:nkb // 2, hh * 128:(hh + 1) * 128])
                nc.sync.dma_start(
                    out=v_sb[:, nkb // 2:nkb, :],
                    in_=v_d_v[b][:, nkb // 2:nkb, hh * 128:(hh + 1) * 128])
                return v_sb

            # ---------------- emit ----------------
            # stage A blocks: batch0 tokens [0,1024), batch1 [0,1024)
            # pass-1 roped k tiles persist in SBUF so pass 2 only ropes
            # the second token half (saves DVE work + ring traffic)
            kp_tiles = {}

            def rope_inplace(jt, nm):
                t = kqp[jt]
                sw = pb.tile([128, TH], bf16, tag="raw", bufs=5,
                             name=f"{nm}_sw")
                nc.sync.dma_start(out=sw[0:64, :], in_=t[64:128, :])
                nc.sync.dma_start(out=sw[64:128, :], in_=t[0:64, :])
                t2 = pb.tile([128, TH], bf16, tag="ropetmp", bufs=2,
                             name=f"{nm}_t2")
                with tc.high_priority():
                    nc.vector.tensor_mul(t2[:], sw[:], sin_sb[:, 0:TH])
                    nc.vector.tensor_mul(t[:], t[:], cos_sb[:, 0:TH])
                    nc.vector.tensor_add(t[:], t[:], t2[:])
                return t

            def pass1_inst(i, tail):
                kT = rope_inplace(i, f"k1_{i}")
                kp_tiles[i] = kT
                qT = load_rope(2 * i + 1, 0, TH, "qr_r", f"q1_{i}")
                v_sb = load_v(i, TH // 128, 0)
                for g in range(2):
                    tail = attn_block(i, g, (kT, None), qT, g * 512, v_sb,
                                      a2a_in[0], 0, tail)
                return tail

            def pass2_loads(i):
                kT2 = load_rope(2 * i, TH, S, "kr_r", f"k2_{i}")
                qT = load_rope(2 * i + 1, TH, S, "qr_r", f"q2_{i}")
                v_sb = load_v(i, S // 128, 1)
                return kT2, qT, v_sb

            def pass2_inst(i, tail, pre=None):
                kT2, qT, v_sb = pre if pre is not None else pass2_loads(i)
                for g in range(2, 4):
                    tail = attn_block(i, g, (kp_tiles[i], kT2), qT,
                                      (g - 2) * 512, v_sb,
                                      a2a_in[1], 1, tail)
                return tail

            at_tags = ["at0", "hs16a"]

            def c_at_alloc(half):
                return pa.tile([128, NJB, QT], bf16, tag=at_tags[half],
                               bufs=1, name=f"at_{half}")

            def c_at_dma(at, half, chunks, eng):
                # the Activation DGE queue is slow (~23 GB/s serial) but
                # idle, so descriptors parked on an unmet dependency block
                # nothing; sync-ring chunks are only emitted once their
                # dependency is certainly met
                for sp in chunks:
                    eng.dma_start(out=at[:, 8 * sp:8 * (sp + 1), :],
                                  in_=at_v[half][:, 8 * sp:8 * (sp + 1), :])

            # wo stream recycles the stage-A weight tags; the jb<16 / jb>=16
            # split means each slot frees at mid-block, giving half a block
            # of load-ahead even with single-buffered tags
            wo_tags = ["wv_a", "wv_b", "w8a", "w8b"]

            def c_wo_load(half, mb):
                tagA = wo_tags[2 * (mb % 2)]
                tagB = wo_tags[2 * (mb % 2) + 1]
                woA = pa.tile([128, 16, 512], bf16, tag=tagA, bufs=1,
                              name=f"woA_{half}_{mb}")
                woB = pa.tile([128, 16, 512], bf16, tag=tagB, bufs=1,
                              name=f"woB_{half}_{mb}")
                nc.sync.dma_start(
                    out=woA[:], in_=woT_v[:, 0:16, mb * 512:(mb + 1) * 512])
                nc.sync.dma_start(
                    out=woB[:], in_=woT_v[:, 16:32, mb * 512:(mb + 1) * 512])
                return woA, woB

            def c_chunks(half, mb, at, woA, woB, quantum=4):
                for t in range(QT // 128):
                    psc = ps.tile([128, 512], f32, tag="psA", bufs=2,
                                  name=f"psC_{half}_{mb}_{t}")
                    for jb0 in range(0, NJB, quantum):
                        for jb in range(jb0, jb0 + quantum):
                            wsrc = woA if jb < 16 else woB
                            nc.tensor.matmul(
                                psc[:],
                                at[:, jb, t * 128:(t + 1) * 128],
                                wsrc[:, jb % 16, :],
                                start=(jb == 0), stop=(jb == NJB - 1))
                        if jb0 + quantum < NJB:
                            yield 'c'
                    oc = pc.tile([128, 512], bf16, tag="oC", bufs=2,
                                 name=f"oC_{half}_{mb}_{t}")
                    nc.scalar.copy(oc[:], psc[:])
                    r0 = half * QT + t * 128
                    for sp in range(2):
                        nc.sync.dma_start(
                            out=out[:][r0:r0 + 128,
                                       mb * 512 + 256 * sp:
                                       mb * 512 + 256 * (sp + 1)],
                            in_=oc[:, 256 * sp:256 * (sp + 1)])
                    yield 'c_cb'

            def c_compute(half, mb, at, woA, woB):
                for _ in c_chunks(half, mb, at, woA, woB):
                    pass

            with nc.named_scope("stageA01"):
                wts = alloc_weights()
                stage_a(0, wts, split=True)
                load_consts()
                for tb in (1, 4, 5):
                    stage_a(tb, wts)
            with nc.named_scope("pass1"):
                # stage-A blocks 2,3 are woven into pass 1's attention
                # stalls (one 4-matmul chunk per score batch); what's left
                # drains densely after the collective is enqueued
                FILL[0] = filler1 = Filler([2, 3], wts)
                tail = None
                for i in range(NI):
                    tail = pass1_inst(i, tail)
                tail()
                FILL[0] = None
            nc.gpsimd.collective_compute(
                "AllToAll", mybir.AluOpType.bypass, replica_groups=GROUPS,
                ins=[a2a_in[0][:]], outs=[a2a_out[0][:]])

            with nc.named_scope("stageA23"):
                filler1.drain()
                # issue pass 2's first loads right after the drain (their
                # kq/v stores are all emitted by now) so the descriptors
                # precede the A2A #1 data phase on the rings
                pre2 = [pass2_loads(0), pass2_loads(1)]
            with nc.named_scope("pass2"):
                # blocks 6,7 fill pass 2's batch-0 instances and must be
                # fully drained before the batch-1 instances read them
                FILL[0] = filler2 = Filler([6, 7], wts)
                tail = None
                for i in range(NI):
                    if i == 0:
                        # at0 has a dedicated buffer (no WAR dependency),
                        # so its slow Act-queue transfer starts right at
                        # pass-2 start and lands well before stage C
                        at0 = c_at_alloc(0)
                        c_at_dma(at0, 0, [0, 1, 2, 3], nc.scalar)
                    if i == 4:
                        FILL[0] = None
                        filler2.drain()
                    if i == 5:
                        # prefetch stage C half-0 weights while pass 2 runs
                        # (safe: buffer and input deps are already met when
                        # the descriptors reach the DMA rings; a DMA whose
                        # deps resolve only after a collective would block
                        # its ring and starve loads queued behind it).
                        # Two blocks are resident before A2A #2's data
                        # phase hogs the rings.
                        wo00 = c_wo_load(0, 0)
                    if i == 6:
                        wo01 = c_wo_load(0, 1)
                    tail = pass2_inst(i, tail,
                                      pre2[i] if i < len(pre2) else None)
                tail()
            nc.gpsimd.collective_compute(
                "AllToAll", mybir.AluOpType.bypass, replica_groups=GROUPS,
                ins=[a2a_in[1][:]], outs=[a2a_out[1][:]])

            # ---------------- stage C: token-quarter o_proj ----------------
            with nc.named_scope("stageC"):
                at1 = c_at_alloc(1)
                # first half parks on the idle Act queue until A2A #2
                c_at_dma(at1, 1, [0, 1], nc.scalar)
                for mb in range(H // 512):
                    if mb == 0:
                        woA, woB = wo00
                    elif mb == 1:
                        woA, woB = wo01
                    else:
                        woA, woB = c_wo_load(0, mb)
                    c_compute(0, mb, at0, woA, woB)
                # by now A2A #2 has certainly landed: the rest of at1 can
                # go on the fast sync rings without parking them
                c_at_dma(at1, 1, [2, 3], nc.sync)
                for mb in range(H // 512):
                    woA, woB = c_wo_load(1, mb)
                    c_compute(1, mb, at1, woA, woB)

    nc.finalize()
    return nc


_NC_CACHE = None


def _get_nc():
    global _NC_CACHE
    if _NC_CACHE is None:
        _NC_CACHE = build_nc()
    return _NC_CACHE


def _host_inputs(hidden_states, positions, w_pack, w_o):
    hidden_states = np.asarray(hidden_states, dtype=np.float32)
    positions = np.asarray(positions)
    w_pack = np.asarray(w_pack, dtype=np.float32)
    w_o = np.asarray(w_o, dtype=np.float32)

    half = HD // 2
    inv_freq = (1.0 / (THETA ** (np.arange(half, dtype=np.float32) / half)))

    # causal mask variants for the 4 diagonal (128x512) tiles of a q-block
    masks = np.empty((4, 128, 512), dtype=np.float32)
    xs = np.arange(512)[None, :]
    ps = np.arange(128)[:, None]
    for v in range(4):
        masks[v] = (xs >= ps + 128 * v).astype(np.float32)

    woT_full = np.ascontiguousarray(w_o.T).astype(ml_dtypes.bfloat16)
    # both batches side by side: [H, B*S]
    hsT = np.concatenate([hidden_states[0].T, hidden_states[1].T], axis=1)
    # blocked [128 p, (tb n), t] layout for contiguous stage-A loads
    hsb = np.ascontiguousarray(
        hsT.reshape(NIB, 128, NTB, TB).transpose(1, 2, 0, 3)
        .reshape(128, NTB * NIB, TB))
    hs16d = hsb.astype(ml_dtypes.bfloat16)
    hs8d = (hsb * SCALEQK).astype(ml_dtypes.float8_e4m3)

    ang = positions[0].astype(np.float32)[None, :] * inv_freq[:, None]
    cos_t = np.cos(ang).astype(np.float32)                 # [64, S]
    sin_t = np.sin(ang).astype(np.float32)
    dsc = 1.0 / (SCALEQK * SCALEQK)
    cosf = (np.concatenate([cos_t, cos_t], axis=0) * dsc) \
        .astype(ml_dtypes.bfloat16)
    sinm = (np.concatenate([-sin_t, sin_t], axis=0) * dsc) \
        .astype(ml_dtypes.bfloat16)
    masks16 = masks.astype(ml_dtypes.bfloat16)

    in_maps = []
    for c in range(NCORES):
        heads = np.arange(HPC * c, HPC * (c + 1))
        kq_parts, v_parts = [], []
        for h in heads:
            hr = np.arange(h * HD, (h + 1) * HD)
            kq_parts += [w_pack[H + hr], w_pack[hr]]       # k then q
            v_parts.append(w_pack[2 * H + hr])
        wT_kq = np.concatenate(kq_parts, axis=0).T             # [H, 2*JCC]
        # SwInterleave stationary: [p, ob*16+i, 256] with col 2c+m =
        # member m's weight column (127-c)
        A = wT_kq.reshape(16, 2, 128, 8, 128)[:, :, :, :, ::-1]
        wT8 = (A.transpose(2, 3, 0, 4, 1).reshape(128, 128, 256)
               * SCALEQK)
        wTv = np.concatenate(v_parts, axis=0).T                # [H, JCC]
        in_maps.append({
            "hs16d": hs16d,
            "hs8d": hs8d,
            "wT8": np.ascontiguousarray(wT8).astype(ml_dtypes.float8_e4m3),
            "wTv": np.ascontiguousarray(wTv).astype(ml_dtypes.bfloat16),
            "woT": woT_full,
            "cosf": cosf,
            "sinm": sinm,
            "masks": masks16,
        })
    return in_maps


def _assemble(results):
    out = np.empty((B, S, H), dtype=np.float32)
    for c in range(NCORES):
        b, q = divmod(c, NCORES // B)
        res = results[c]["out"]                    # [2*QT, H]
        out[b][QT * q:QT * (q + 1)] = res[:QT]
        out[b][TH + QT * q:TH + QT * (q + 1)] = res[QT:]
    return out


def kernel(hidden_states, positions, w_pack, w_o):
    import os
    os.environ["BASS_NEVER_TRACE"] = "1"
    nc = _get_nc()
    in_maps = _host_inputs(hidden_states, positions, w_pack, w_o)
    res = run_bass_kernel_spmd(nc, in_maps, list(range(NCORES)))
    return _assemble(res.results)
